# revision 32
# baseline (speedup 1.0000x reference)
"""Causal linear attention (elu+1 feature map) on 8 Trainium2 NeuronCores.

Full inputs (n=2, l=2048, h=8, d=64) fp32 are sharded over the 16 (n,h)
head-sequences: core i handles pairs (2i, 2i+1). The elu(x)+1 feature maps
and all layout shuffles run on the HOST (numpy); the device does only the
memory/compute-heavy chunked causal scan (chunk C=128, state stride 2).

Per scan step s (chunks c0=2s, c1=2s+1), with Kf/Qf host-fmapped:

  at_ps = [AT(c0) p0|p1 | CROSS p0|p1 | AT(c1) p0|p1]     (2 matmuls: the
          c-major blocked qfb makes [AT(c0)|CROSS] one 512-col moving)
  atm   = tri-mask(at blocks 0,1,4,5 via one broadcast-mask DVE op)
          + CROSS copied by ScalarE
  out(c) = atm(c)^T @ Vaug_c (+ CROSS^T @ Vaug_c0 for c1)
           + per-pair qfb_c^T @ S_sb                      (4 snap matmuls)
  S_ps  += Kf_c^T @ Vaug_c  (both chunks, PSUM fp32, serial accumulator)

S_sb is a single full f16 ScalarE copy of S_ps per step (ScalarE on
purpose -- DVE reads of the PE-accumulated S bank hang). Its cross-pair
garbage blocks are killed by the zero off-pair rows of the blocked qfb
stationary, so no zeroing or dense qfT upload is needed.

The run is DMA-stream-bound: the two HWDGE rings share 16 SDMA engines
(~240 GB/s aggregate pre-compute, ~4x slower once compute contends for
SBUF ports), so bytes are everything. The PE allows mixed input dtypes
(only fp32 must not mix), so all stationaries on the attention/state
path (qfb, kfT, kf) are fp8 e4m3 while the value/moving side stays f16:
1.58 MB total input. fp8 errors in the shared qf*kf weights mostly
cancel between numerator and denominator. Every DMA gets its own SBUF
tile (dep tracking is tile-granular), pieces sized/ordered by first use.

PSUM start=True is used on the first matmul touching each bank per group
(has_written semantics: later disjoint writers store, overlapping ones
accumulate) -- no zero-init matmuls. Out is written as f16 (num|den)/16;
the host does the divide.

Host layouts (DMAs contiguous):
  qfb  f8 : (128, 4096)  [(64p' + d), (256c + 128p + i)], zero unless p'=p
  kfT  f8 : (128, 2048)  [(64p + d), (128c + i)]
  kf8  f8 : (128, 2048)  [i, (128c + 64p + d)]
  vb   f16: (128, 2080)  [i, (130c + 65p + x)], x=64 -> 1
  mz   f16: (128, 128)   tri mask (j<=i)
  o    f16: (128, 2080)  [i, (130c + 65p + x)]  (num/16 | den/16)
"""
import numpy as np
import ml_dtypes
from contextlib import ExitStack

import concourse.bacc as bacc
import concourse.bass as bass
import concourse.tile as tile
from concourse import mybir
from concourse.bass_utils import run_bass_kernel_spmd

N, L, H, D = 2, 2048, 8, 64
C = 128                 # chunk length
NCH = L // C            # 16 chunks
PAIRS = 2
QW = NCH * C            # 2048 cols (transposed layouts)
BW = NCH * PAIRS * C    # 4096 blocked qfb cols
SW = PAIRS * (D + 1)    # 130: state cols [S_p0 | ksum_p0 | S_p1 | ksum_p1]
ATW = 6 * C             # at: [ATc0 p0|ATc0 p1|CROSS p0|CROSS p1|ATc1 p0|p1]
OW = NCH * SW           # 2080 output cols
OSCALE = 1.0 / 16.0     # keeps num/den inside f16 range

f16 = mybir.dt.float16
f32 = mybir.dt.float32
f8 = mybir.dt.float8e4
np_f8 = ml_dtypes.float8_e4m3fn
OP = mybir.AluOpType


def build_kernel():
    nc = bacc.Bacc("TRN2", target_bir_lowering=False, debug=False, num_devices=8)
    qfb_d = nc.dram_tensor("qfb", (C, BW), f8, kind="ExternalInput").ap()
    kfT_d = nc.dram_tensor("kfT", (C, QW), f8, kind="ExternalInput").ap()
    kf8_d = nc.dram_tensor("kf8", (C, QW), f8, kind="ExternalInput").ap()
    vb_d = nc.dram_tensor("vb", (C, OW), f16, kind="ExternalInput").ap()
    mz_d = nc.dram_tensor("mz", (C, C), f16, kind="ExternalInput").ap()
    o_d = nc.dram_tensor("o", (C, OW), f16, kind="ExternalOutput").ap()

    with tile.TileContext(nc) as tc, ExitStack() as ctx:
        consts = ctx.enter_context(tc.tile_pool(name="consts", bufs=1))
        sm_pool = ctx.enter_context(tc.tile_pool(name="sm", bufs=2))
        at_psum = ctx.enter_context(tc.tile_pool(name="at", bufs=2, space="PSUM"))
        out_psum = ctx.enter_context(tc.tile_pool(name="out", bufs=3, space="PSUM"))
        s_psum = ctx.enter_context(tc.tile_pool(name="sp", bufs=1, space="PSUM"))

        # one SBUF tile per DMA (dep tracking is tile-granular)
        qfb0 = consts.tile([C, BW // 4], f8, tag="qfb0")
        qfb1 = consts.tile([C, BW // 4], f8, tag="qfb1")
        qfb2 = consts.tile([C, BW // 4], f8, tag="qfb2")
        qfb3 = consts.tile([C, BW // 4], f8, tag="qfb3")
        kfT0 = consts.tile([C, QW // 4], f8, tag="kfT0")
        kfT1 = consts.tile([C, QW // 4], f8, tag="kfT1")
        kfTb = consts.tile([C, QW // 2], f8, tag="kfTb")
        kf80 = consts.tile([C, QW // 4], f8, tag="kf80")
        kf81 = consts.tile([C, QW // 4], f8, tag="kf81")
        kf8b = consts.tile([C, QW // 2], f8, tag="kf8b")
        vb0 = consts.tile([C, OW // 4], f16, tag="vb0")
        vb1 = consts.tile([C, OW // 4], f16, tag="vb1")
        vbb = consts.tile([C, OW // 2], f16, tag="vbb")
        maskt = consts.tile([C, C], f16)
        sb0 = consts.tile([C, SW], f16, tag="sb0")  # fully overwritten
        sb1 = consts.tile([C, SW], f16, tag="sb1")  # each step
        sbs = [sb0, sb1]
        ob = consts.tile([C, OW], f16)              # output staging

        def _qfb_piece(c):
            return [qfb0, qfb1, qfb2, qfb3][c // 4], c % 4

        def qfb_mv(c, w):            # at moving: chunks c..c+w-1 blocks
            t, lc = _qfb_piece(c)
            return t[:, lc * 2 * C:(lc + w) * 2 * C]

        def qfb_st(c, p):            # snap stationary (zero off-pair rows)
            t, lc = _qfb_piece(c)
            return t[:, lc * 2 * C + p * C:lc * 2 * C + (p + 1) * C]

        def kfT_st(c):               # at stationary
            t = [kfT0, kfT1, kfTb, kfTb][c // 4]
            lc = c % 4 if c < 8 else c - 8
            return t[:, lc * C:(lc + 1) * C]

        def kf_st(c):                # S-update stationary (fp8)
            t = [kf80, kf81, kf8b, kf8b][c // 4]
            lc = c % 4 if c < 8 else c - 8
            return t[:, lc * C:(lc + 1) * C]

        def vb_mv(c, p, w):          # vaug moving (w cols from pair p)
            t = [vb0, vb1, vbb, vbb][c // 4]
            lc = c % 4 if c < 8 else c - 8
            base = lc * SW + p * (D + 1)
            return t[:, base:base + w]

        # input DMAs on the two HWDGE rings, ordered by first use
        nc.sync.dma_start(kfT0, kfT_d[:, 0:QW // 4])
        nc.sync.dma_start(qfb0, qfb_d[:, 0:BW // 4])
        nc.sync.dma_start(kfT1, kfT_d[:, QW // 4:QW // 2])
        nc.sync.dma_start(qfb1, qfb_d[:, BW // 4:BW // 2])
        nc.sync.dma_start(kfTb, kfT_d[:, QW // 2:QW])
        nc.sync.dma_start(qfb2, qfb_d[:, BW // 2:3 * BW // 4])
        nc.sync.dma_start(qfb3, qfb_d[:, 3 * BW // 4:BW])
        nc.scalar.dma_start(maskt, mz_d)
        nc.scalar.dma_start(kf80, kf8_d[:, 0:QW // 4])
        nc.scalar.dma_start(vb0, vb_d[:, 0:OW // 4])
        nc.scalar.dma_start(kf81, kf8_d[:, QW // 4:QW // 2])
        nc.scalar.dma_start(vb1, vb_d[:, OW // 4:OW // 2])
        nc.scalar.dma_start(kf8b, kf8_d[:, QW // 2:QW])
        nc.scalar.dma_start(vbb, vb_d[:, OW // 2:OW])

        # running state accumulator (off-diagonal blocks hold unread garbage)
        S_ps = s_psum.tile([C, SW], f32)

        def emit_at(s):
            """at matmuls + tri mask + cross copy for step s; returns atm."""
            c0, c1 = 2 * s, 2 * s + 1
            at_ps = at_psum.tile([C, ATW], f32, tag="at")
            atm = sm_pool.tile([C, ATW], f16, tag="atm")
            mask_b = bass.AP(tensor=maskt.tensor, offset=maskt.offset,
                             ap=[list(maskt.ap[0]), [0, 2], [1, C]])
            a3 = at_ps.rearrange("i (b x) -> i b x", b=6)
            m3 = atm.rearrange("i (b x) -> i b x", b=6)
            # tri-masking split in two so atm trails each at matmul closely
            nc.tensor.matmul(at_ps[:, 0:4 * C], kfT_st(c0), qfb_mv(c0, 2),
                             start=True, stop=True)
            nc.vector.tensor_tensor(out=m3[:, 0:2], in0=a3[:, 0:2],
                                    in1=mask_b, op=OP.mult)
            nc.scalar.copy(atm[:, 2 * C:4 * C], at_ps[:, 2 * C:4 * C])
            nc.tensor.matmul(at_ps[:, 4 * C:6 * C], kfT_st(c1), qfb_mv(c1, 1),
                             start=True, stop=True)
            nc.vector.tensor_tensor(out=m3[:, 4:6], in0=a3[:, 4:6],
                                    in1=mask_b, op=OP.mult)
            return atm

        atm = emit_at(0)
        for s in range(8):
            c0, c1 = 2 * s, 2 * s + 1
            out_ps = out_psum.tile([C, 2 * SW], f32, tag="out")

            # state updates first (skipped once no later chunk needs them):
            # putting them ahead of the snap matmuls lets the next step's
            # ScalarE snapshot copy start a half-step earlier
            for c in (c0, c1):
                if c <= NCH - 3:
                    nc.tensor.matmul(
                        S_ps, kf_st(c), vb_mv(c, 0, SW),
                        start=(c == 0), stop=(c == NCH - 3),
                        skip_group_check=True)

            # inter-chunk terms from the snapshot, per (chunk, pair):
            # the qfb stationary's zero rows kill the snapshot garbage
            if s > 0:
                sb = sbs[s % 2]
                for dj, c in ((0, c0), (1, c1)):
                    for p in range(PAIRS):
                        lo = dj * SW + p * (D + 1)
                        nc.tensor.matmul(
                            out_ps[:, lo:lo + D + 1], qfb_st(c, p),
                            sb[:, p * (D + 1):(p + 1) * (D + 1)],
                            start=(dj == 0 and p == 0), stop=False,
                            skip_group_check=True)

            # f16 state snapshot for step s+1: ONE full copy, garbage and all
            if s < 7:
                nc.scalar.copy(sbs[(s + 1) % 2], S_ps)

            # next step's at matmuls fill PE while DVE masks this step
            atm_next = emit_at(s + 1) if s < 7 else None

            # intra-chunk + cross contributions
            for p in range(PAIRS):
                vs = slice(p * (D + 1), (p + 1) * (D + 1))
                nc.tensor.matmul(        # intra c0
                    out_ps[:, vs], atm[:, p * C:(p + 1) * C],
                    vb_mv(c0, p, D + 1),
                    start=(s == 0 and p == 0), stop=False,
                    skip_group_check=True)
            for p in range(PAIRS):
                vs = slice(SW + p * (D + 1), SW + (p + 1) * (D + 1))
                nc.tensor.matmul(        # cross -> c1
                    out_ps[:, vs], atm[:, (2 + p) * C:(3 + p) * C],
                    vb_mv(c0, p, D + 1),
                    start=False, stop=False, skip_group_check=True)
            for p in range(PAIRS):
                vs = slice(SW + p * (D + 1), SW + (p + 1) * (D + 1))
                nc.tensor.matmul(        # intra c1
                    out_ps[:, vs], atm[:, (4 + p) * C:(5 + p) * C],
                    vb_mv(c1, p, D + 1),
                    start=False, stop=(p == PAIRS - 1),
                    skip_group_check=True)

            # scaled f16 staging copy; host does the divide
            nc.vector.tensor_scalar_mul(
                ob[:, s * 2 * SW:(s + 1) * 2 * SW], out_ps, OSCALE)
            nc.sync.dma_start(o_d[:, s * 2 * SW:(s + 1) * 2 * SW],
                              ob[:, s * 2 * SW:(s + 1) * 2 * SW])
            atm = atm_next

    nc.compile()
    return nc


_nc_cache = None


def _get_nc():
    global _nc_cache
    if _nc_cache is None:
        _nc_cache = build_kernel()
    return _nc_cache


def _fmap_np(x):
    # elu(x) + 1 in fp32 on host
    return np.where(x < 0.0, np.exp(np.minimum(x, 0.0)), x + 1.0)


def _core_pairs(x, core):
    flat = np.asarray(x).transpose(0, 2, 1, 3).reshape(N * H, L, D)
    return flat[2 * core:2 * core + 2]          # (2, L, D) fp32


def _t_layout(xc, dtype):
    # (2, L, D) -> (128, 2048) [(64p + d), (128c + i)]
    return np.ascontiguousarray(
        xc.reshape(PAIRS, NCH, C, D).transpose(0, 3, 1, 2).reshape(C, QW)
    ).astype(dtype)


def make_in_maps(queries, keys, values):
    mz = np.triu(np.ones((C, C), np.float16))   # mask[j,i]=1 iff j<=i
    in_maps = []
    for core in range(8):
        qf = _fmap_np(_core_pairs(queries, core).astype(np.float32))
        kf = _fmap_np(_core_pairs(keys, core).astype(np.float32))
        vc = _core_pairs(values, core).astype(np.float32)

        # blocked qfb, c-major: [(64p'+d), (256c+128p+i)], zero unless p'=p
        qft = qf.reshape(PAIRS, NCH, C, D).astype(np_f8)  # (p,c,i,d)
        qfb = np.zeros((C, NCH, PAIRS, C), np_f8)  # (row, c, p, i)
        for p in range(PAIRS):
            qfb[p * D:(p + 1) * D, :, p, :] = qft[p].transpose(2, 0, 1)
        qfb = qfb.reshape(C, BW)

        kf8 = kf.reshape(PAIRS, NCH, C, D).transpose(2, 1, 0, 3) \
                .reshape(C, NCH * PAIRS * D).astype(np_f8)
        va = np.ones((PAIRS, NCH, C, D + 1), np.float32)
        va[..., 0:D] = vc.reshape(PAIRS, NCH, C, D)
        vb = va.transpose(2, 1, 0, 3).reshape(C, OW).astype(np.float16)
        in_maps.append({
            "qfb": np.ascontiguousarray(qfb),
            "kfT": _t_layout(kf, np_f8),
            "kf8": np.ascontiguousarray(kf8),
            "vb": np.ascontiguousarray(vb),
            "mz": mz,
        })
    return in_maps


def _unpack_out(o_arr):
    # (128, 2080) f16 (num|den)/16 -> (2, L, D) fp32 normalized
    o4 = o_arr.astype(np.float32).reshape(C, NCH, PAIRS, D + 1)
    res = o4[..., 0:D] / o4[..., D:D + 1]
    return res.transpose(2, 1, 0, 3).reshape(PAIRS, L, D)


def kernel(queries, keys, values):
    nc = _get_nc()
    in_maps = make_in_maps(queries, keys, values)
    res = run_bass_kernel_spmd(nc, in_maps, core_ids=list(range(8)))
    out = np.zeros((N, L, H, D), np.float32)
    for core in range(8):
        oc = _unpack_out(res.results[core]["o"])
        for p in range(PAIRS):
            flat = 2 * core + p
            out[flat // H, :, flat % H, :] = oc[p]
    return out


# revision 34
# speedup vs baseline: 1.0230x; 1.0230x over previous
"""Causal linear attention (elu+1 feature map) on 8 Trainium2 NeuronCores.

Full inputs (n=2, l=2048, h=8, d=64) fp32 are sharded over the 16 (n,h)
head-sequences: core i handles pairs (2i, 2i+1). The elu(x)+1 feature maps
and all layout shuffles run on the HOST (numpy); the device does only the
memory/compute-heavy chunked causal scan (chunk C=128, state stride 2).

Per scan step s (chunks c0=2s, c1=2s+1), with Kf/Qf host-fmapped:

  at_ps = [AT(c0) p0|p1 | CROSS p0|p1 | AT(c1) p0|p1]     (2 matmuls: the
          c-major blocked qfb makes [AT(c0)|CROSS] one 512-col moving)
  atm   = tri-mask(at blocks 0,1,4,5 via one broadcast-mask DVE op)
          + CROSS copied by ScalarE
  out(c) = atm(c)^T @ Vaug_c (+ CROSS^T @ Vaug_c0 for c1)
           + per-pair qfb_c^T @ S_sb                      (4 snap matmuls)
  S_ps  += Kf_c^T @ Vaug_c  (both chunks, PSUM fp32, serial accumulator)

S_sb is a single full f16 ScalarE copy of S_ps per step (ScalarE on
purpose -- DVE reads of the PE-accumulated S bank hang). Its cross-pair
garbage blocks are killed by the zero off-pair rows of the blocked qfb
stationary, so no zeroing or dense qfT upload is needed.

The run is DMA-stream-bound: the two HWDGE rings share 16 SDMA engines
(~240 GB/s aggregate pre-compute, ~4x slower once compute contends for
SBUF ports), so bytes are everything. The PE allows mixed input dtypes
(only fp32 must not mix), so all stationaries on the attention/state
path (qfb, kfT, kf) are fp8 e4m3 while the value/moving side stays f16:
1.58 MB total input. fp8 errors in the shared qf*kf weights mostly
cancel between numerator and denominator. Every DMA gets its own SBUF
tile (dep tracking is tile-granular), pieces sized/ordered by first use.

PSUM start=True is used on the first matmul touching each bank per group
(has_written semantics: later disjoint writers store, overlapping ones
accumulate) -- no zero-init matmuls. Out is written as f16 (num|den)/16;
the host does the divide.

Host layouts (DMAs contiguous):
  qfb  f8 : (128, 4096)  [(64p' + d), (256c + 128p + i)], zero unless p'=p
  kfT  f8 : (128, 2048)  [(64p + d), (128c + i)]
  kf8  f8 : (128, 2048)  [i, (128c + 64p + d)]
  vb   f16: (128, 2080)  [i, (130c + 65p + x)], x=64 -> 1
  mz   f16: (128, 128)   tri mask (j<=i)
  o    f16: (128, 2080)  [i, (130c + 65p + x)]  (num/16 | den/16)
"""
import numpy as np
import ml_dtypes
from contextlib import ExitStack

import concourse.bacc as bacc
import concourse.bass as bass
import concourse.tile as tile
from concourse import mybir
from concourse.bass_utils import run_bass_kernel_spmd

N, L, H, D = 2, 2048, 8, 64
C = 128                 # chunk length
NCH = L // C            # 16 chunks
PAIRS = 2
QW = NCH * C            # 2048 cols (transposed layouts)
BW = NCH * PAIRS * C    # 4096 blocked qfb cols
SW = PAIRS * (D + 1)    # 130: state cols [S_p0 | ksum_p0 | S_p1 | ksum_p1]
ATW = 6 * C             # at: [ATc0 p0|ATc0 p1|CROSS p0|CROSS p1|ATc1 p0|p1]
OW = NCH * SW           # 2080 output cols
OSCALE = 1.0 / 16.0     # keeps num/den inside f16 range

f16 = mybir.dt.float16
f32 = mybir.dt.float32
f8 = mybir.dt.float8e4
np_f8 = ml_dtypes.float8_e4m3fn
OP = mybir.AluOpType


def build_kernel():
    nc = bacc.Bacc("TRN2", target_bir_lowering=False, debug=False, num_devices=8)
    qfb_d = nc.dram_tensor("qfb", (C, BW), f8, kind="ExternalInput").ap()
    kfT_d = nc.dram_tensor("kfT", (C, QW), f8, kind="ExternalInput").ap()
    kf8_d = nc.dram_tensor("kf8", (C, QW), f8, kind="ExternalInput").ap()
    vb_d = nc.dram_tensor("vb", (C, OW), f16, kind="ExternalInput").ap()
    mz_d = nc.dram_tensor("mz", (C, C), f16, kind="ExternalInput").ap()
    o_d = nc.dram_tensor("o", (C, OW), f16, kind="ExternalOutput").ap()

    with tile.TileContext(nc) as tc, ExitStack() as ctx:
        consts = ctx.enter_context(tc.tile_pool(name="consts", bufs=1))
        sm_pool = ctx.enter_context(tc.tile_pool(name="sm", bufs=2))
        at_psum = ctx.enter_context(tc.tile_pool(name="at", bufs=2, space="PSUM"))
        out_psum = ctx.enter_context(tc.tile_pool(name="out", bufs=3, space="PSUM"))
        s_psum = ctx.enter_context(tc.tile_pool(name="sp", bufs=1, space="PSUM"))

        # one SBUF tile per DMA (dep tracking is tile-granular)
        qfb0 = consts.tile([C, BW // 4], f8, tag="qfb0")
        qfb1 = consts.tile([C, BW // 4], f8, tag="qfb1")
        qfb2 = consts.tile([C, BW // 4], f8, tag="qfb2")
        qfb3 = consts.tile([C, BW // 4], f8, tag="qfb3")
        kfT0 = consts.tile([C, QW // 4], f8, tag="kfT0")
        kfT1 = consts.tile([C, QW // 4], f8, tag="kfT1")
        kfTb = consts.tile([C, QW // 2], f8, tag="kfTb")
        kf80 = consts.tile([C, QW // 4], f8, tag="kf80")
        kf81 = consts.tile([C, QW // 4], f8, tag="kf81")
        kf8b = consts.tile([C, QW // 2], f8, tag="kf8b")
        vb0 = consts.tile([C, OW // 4], f16, tag="vb0")
        vb1 = consts.tile([C, OW // 4], f16, tag="vb1")
        vbb = consts.tile([C, OW // 2], f16, tag="vbb")
        maskt = consts.tile([C, C], f16)
        sb0 = consts.tile([C, SW], f16, tag="sb0")  # fully overwritten
        sb1 = consts.tile([C, SW], f16, tag="sb1")  # each step
        sbs = [sb0, sb1]
        ob = consts.tile([C, OW], f16)              # output staging

        def _qfb_piece(c):
            return [qfb0, qfb1, qfb2, qfb3][c // 4], c % 4

        def qfb_mv(c, w):            # at moving: chunks c..c+w-1 blocks
            t, lc = _qfb_piece(c)
            return t[:, lc * 2 * C:(lc + w) * 2 * C]

        def qfb_st(c, p):            # snap stationary (zero off-pair rows)
            t, lc = _qfb_piece(c)
            return t[:, lc * 2 * C + p * C:lc * 2 * C + (p + 1) * C]

        def kfT_st(c):               # at stationary
            t = [kfT0, kfT1, kfTb, kfTb][c // 4]
            lc = c % 4 if c < 8 else c - 8
            return t[:, lc * C:(lc + 1) * C]

        def kf_st(c):                # S-update stationary (fp8)
            t = [kf80, kf81, kf8b, kf8b][c // 4]
            lc = c % 4 if c < 8 else c - 8
            return t[:, lc * C:(lc + 1) * C]

        def vb_mv(c, p, w):          # vaug moving (w cols from pair p)
            t = [vb0, vb1, vbb, vbb][c // 4]
            lc = c % 4 if c < 8 else c - 8
            base = lc * SW + p * (D + 1)
            return t[:, base:base + w]

        # input DMAs on the two HWDGE rings, ordered by first use
        nc.sync.dma_start(kfT0, kfT_d[:, 0:QW // 4])
        nc.sync.dma_start(qfb0, qfb_d[:, 0:BW // 4])
        nc.sync.dma_start(kfT1, kfT_d[:, QW // 4:QW // 2])
        nc.sync.dma_start(qfb1, qfb_d[:, BW // 4:BW // 2])
        nc.sync.dma_start(kfTb, kfT_d[:, QW // 2:QW])
        nc.sync.dma_start(qfb2, qfb_d[:, BW // 2:3 * BW // 4])
        nc.sync.dma_start(qfb3, qfb_d[:, 3 * BW // 4:BW])
        nc.scalar.dma_start(maskt, mz_d)
        nc.scalar.dma_start(kf80, kf8_d[:, 0:QW // 4])
        nc.scalar.dma_start(vb0, vb_d[:, 0:OW // 4])
        nc.scalar.dma_start(kf81, kf8_d[:, QW // 4:QW // 2])
        nc.scalar.dma_start(vb1, vb_d[:, OW // 4:OW // 2])
        nc.scalar.dma_start(kf8b, kf8_d[:, QW // 2:QW])
        nc.scalar.dma_start(vbb, vb_d[:, OW // 2:OW])

        # running state accumulator (off-diagonal blocks hold unread garbage)
        S_ps = s_psum.tile([C, SW], f32)

        def emit_at(s):
            """at matmuls + tri mask + cross copy for step s; returns atm."""
            c0, c1 = 2 * s, 2 * s + 1
            at_ps = at_psum.tile([C, ATW], f32, tag="at")
            atm = sm_pool.tile([C, ATW], f16, tag="atm")
            nc.tensor.matmul(at_ps[:, 0:4 * C], kfT_st(c0), qfb_mv(c0, 2),
                             start=True, stop=True)
            nc.tensor.matmul(at_ps[:, 4 * C:6 * C], kfT_st(c1), qfb_mv(c1, 1),
                             start=True, stop=True)
            # tri-mask blocks {0,1,4,5} in one op: broadcast 128x128 mask
            tri_in = bass.AP(tensor=at_ps.tensor, offset=at_ps.offset,
                             ap=[list(at_ps.ap[0]), [4 * C, 2], [C, 2], [1, C]])
            tri_out = bass.AP(tensor=atm.tensor, offset=atm.offset,
                              ap=[list(atm.ap[0]), [4 * C, 2], [C, 2], [1, C]])
            mask_b = bass.AP(tensor=maskt.tensor, offset=maskt.offset,
                             ap=[list(maskt.ap[0]), [0, 2], [0, 2], [1, C]])
            nc.vector.tensor_tensor(out=tri_out, in0=tri_in, in1=mask_b,
                                    op=OP.mult)
            nc.scalar.copy(atm[:, 2 * C:4 * C], at_ps[:, 2 * C:4 * C])
            return atm

        atm = emit_at(0)
        for s in range(8):
            c0, c1 = 2 * s, 2 * s + 1
            out_ps = out_psum.tile([C, 2 * SW], f32, tag="out")

            # state updates first (skipped once no later chunk needs them):
            # putting them ahead of the snap matmuls lets the next step's
            # ScalarE snapshot copy start a half-step earlier
            for c in (c0, c1):
                if c <= NCH - 3:
                    nc.tensor.matmul(
                        S_ps, kf_st(c), vb_mv(c, 0, SW),
                        start=(c == 0), stop=(c == NCH - 3),
                        skip_group_check=True)

            # inter-chunk terms from the snapshot, per (chunk, pair):
            # the qfb stationary's zero rows kill the snapshot garbage
            if s > 0:
                sb = sbs[s % 2]
                for dj, c in ((0, c0), (1, c1)):
                    for p in range(PAIRS):
                        lo = dj * SW + p * (D + 1)
                        nc.tensor.matmul(
                            out_ps[:, lo:lo + D + 1], qfb_st(c, p),
                            sb[:, p * (D + 1):(p + 1) * (D + 1)],
                            start=(dj == 0 and p == 0), stop=False,
                            skip_group_check=True)

            # f16 state snapshot for step s+1: ONE full copy, garbage and all
            if s < 7:
                nc.scalar.copy(sbs[(s + 1) % 2], S_ps)

            # next step's at matmuls fill PE while DVE masks this step
            atm_next = emit_at(s + 1) if s < 7 else None

            # intra-chunk + cross contributions
            for p in range(PAIRS):
                vs = slice(p * (D + 1), (p + 1) * (D + 1))
                nc.tensor.matmul(        # intra c0
                    out_ps[:, vs], atm[:, p * C:(p + 1) * C],
                    vb_mv(c0, p, D + 1),
                    start=(s == 0 and p == 0), stop=False,
                    skip_group_check=True)
            for p in range(PAIRS):
                vs = slice(SW + p * (D + 1), SW + (p + 1) * (D + 1))
                nc.tensor.matmul(        # cross -> c1
                    out_ps[:, vs], atm[:, (2 + p) * C:(3 + p) * C],
                    vb_mv(c0, p, D + 1),
                    start=False, stop=False, skip_group_check=True)
            for p in range(PAIRS):
                vs = slice(SW + p * (D + 1), SW + (p + 1) * (D + 1))
                nc.tensor.matmul(        # intra c1
                    out_ps[:, vs], atm[:, (4 + p) * C:(5 + p) * C],
                    vb_mv(c1, p, D + 1),
                    start=False, stop=(p == PAIRS - 1),
                    skip_group_check=True)

            # scaled f16 staging copy; host does the divide
            nc.vector.tensor_scalar_mul(
                ob[:, s * 2 * SW:(s + 1) * 2 * SW], out_ps, OSCALE)
            if s % 2 == 1:
                k = s // 2
                nc.sync.dma_start(o_d[:, k * 4 * SW:(k + 1) * 4 * SW],
                                  ob[:, k * 4 * SW:(k + 1) * 4 * SW])
            atm = atm_next

    nc.compile()
    return nc


_nc_cache = None


def _get_nc():
    global _nc_cache
    if _nc_cache is None:
        _nc_cache = build_kernel()
    return _nc_cache


def _fmap_np(x):
    # elu(x) + 1 in fp32 on host
    return np.where(x < 0.0, np.exp(np.minimum(x, 0.0)), x + 1.0)


def _core_pairs(x, core):
    flat = np.asarray(x).transpose(0, 2, 1, 3).reshape(N * H, L, D)
    return flat[2 * core:2 * core + 2]          # (2, L, D) fp32


def _t_layout(xc, dtype):
    # (2, L, D) -> (128, 2048) [(64p + d), (128c + i)]
    return np.ascontiguousarray(
        xc.reshape(PAIRS, NCH, C, D).transpose(0, 3, 1, 2).reshape(C, QW)
    ).astype(dtype)


def make_in_maps(queries, keys, values):
    mz = np.triu(np.ones((C, C), np.float16))   # mask[j,i]=1 iff j<=i
    in_maps = []
    for core in range(8):
        qf = _fmap_np(_core_pairs(queries, core).astype(np.float32))
        kf = _fmap_np(_core_pairs(keys, core).astype(np.float32))
        vc = _core_pairs(values, core).astype(np.float32)

        # blocked qfb, c-major: [(64p'+d), (256c+128p+i)], zero unless p'=p
        qft = qf.reshape(PAIRS, NCH, C, D).astype(np_f8)  # (p,c,i,d)
        qfb = np.zeros((C, NCH, PAIRS, C), np_f8)  # (row, c, p, i)
        for p in range(PAIRS):
            qfb[p * D:(p + 1) * D, :, p, :] = qft[p].transpose(2, 0, 1)
        qfb = qfb.reshape(C, BW)

        kf8 = kf.reshape(PAIRS, NCH, C, D).transpose(2, 1, 0, 3) \
                .reshape(C, NCH * PAIRS * D).astype(np_f8)
        va = np.ones((PAIRS, NCH, C, D + 1), np.float32)
        va[..., 0:D] = vc.reshape(PAIRS, NCH, C, D)
        vb = va.transpose(2, 1, 0, 3).reshape(C, OW).astype(np.float16)
        in_maps.append({
            "qfb": np.ascontiguousarray(qfb),
            "kfT": _t_layout(kf, np_f8),
            "kf8": np.ascontiguousarray(kf8),
            "vb": np.ascontiguousarray(vb),
            "mz": mz,
        })
    return in_maps


def _unpack_out(o_arr):
    # (128, 2080) f16 (num|den)/16 -> (2, L, D) fp32 normalized
    o4 = o_arr.astype(np.float32).reshape(C, NCH, PAIRS, D + 1)
    res = o4[..., 0:D] / o4[..., D:D + 1]
    return res.transpose(2, 1, 0, 3).reshape(PAIRS, L, D)


def kernel(queries, keys, values):
    nc = _get_nc()
    in_maps = make_in_maps(queries, keys, values)
    res = run_bass_kernel_spmd(nc, in_maps, core_ids=list(range(8)))
    out = np.zeros((N, L, H, D), np.float32)
    for core in range(8):
        oc = _unpack_out(res.results[core]["o"])
        for p in range(PAIRS):
            flat = 2 * core + p
            out[flat // H, :, flat % H, :] = oc[p]
    return out
